# revision 28
# baseline (speedup 1.0000x reference)
"""Distributed FFT (N = 2^24 complex points) on 8 Trainium2 NeuronCores.

Four-step (Cooley-Tukey) decomposition N = 4096 x 4096:
  launch 1: per global column j1g, FFT_4096 over j2g      (batch parallel over j1g)
  host:     global twiddle wN^{j1g*k2g} + transpose exchange
  launch 2: per global row k2g, FFT_4096 over j1g         (batch parallel over k2g)

Each local FFT_4096 = radix-32 stage A fused with its inter-stage transpose,
then a radix-128 stage B with per-kap2 twiddle-folded weights.

Stage A is a ONE-PASS complex matmul: the contraction axis K = 128 packs
(u-batch-bit x plane x j2) and the moving weight W1 is the 2x2 real
representation of W32, block-diagonal over u. Each data-stationary matmul
(lhsT = a [128, 128] input slice covering 2 signals) emits both output planes
in a single 128-column stream, halving stage-A PE time vs the 2-pass form.
Stage B stays 2-pass (the plane axis leaves the partition dim after stage A -
a lane-locked-evacuation parity constraint makes consecutive 1-pass complex
stages impossible), giving 98304 PE streaming cycles/launch vs 131072 before.

Stage-B weights: B_kap2[j1,k1] = W4096[j1, kap2+32*k1], so {Br, Bi} ship as
one table w2 (2.1MB, kap2-blocked) and -Bi is generated on device by cheap
DVE/ACT negates as each chunk lands (the BIR verifier forbids negative-stride
stationaries, so the reversed-view trick is out). That kills the redundant
third matrix of the old 3.1MB layout and lets w2 stream AFTER the input
ladder, in 8 chunks that each unblock 4 kap2 iterations.

PSUM evacuations rotate over DVE / Activation (the only PSUM-capable
engines; error-diffusion weighted by per-copy time) so neither engine's copy
throughput (~1 col/cycle at 0.96-1.2 GHz) gates the PE (2.4 GHz). PSUM tiles
span TWO banks ([128, 1024]) so one evacuation drains two stage-A groups or
a whole kap2's (re | im) pair, halving the per-instruction PSUM-access
overhead on the binding evac engines; the 8 banks rotate as 4 such tiles.

DMA: all queues serialize at 360 GB/s in the cost model, so bytes are the
floor. Launch-2's input (the twiddled intermediate, host-rescaled) ships as
float8_e3m4 (1.33% rms quantization, measured; total error ~1.4% vs the 2e-2
gate), halving that launch's input traffic. Launch-1 input and all weights
stay bf16. Outputs stay bf16.

Layouts (per launch, per core, batch 512 signals of 4096 points):
  f = j1 + 128*j2; k = kap2 + 32*kap1; b = 2*m + u (m in [0,256)).
  in  [p, m, j1]        p = u*64 + pl*32 + j2
  T   [j1, g, dm, q, kap2, u]   (g, dm) = m split 64x4; stage-B slice is
                                 b-ordered: psum cols (g, dm, u) = b
  out [kap2, k1, pl, b]
"""
import numpy as np
import ml_dtypes

import concourse.mybir as mybir
import concourse.tile as tile
from concourse import bacc
from concourse.bass_utils import run_bass_kernel_spmd

NG = 4096                 # global matrix dimension; N = NG*NG
N = NG * NG
NCORES = 8
BPC = NG // NCORES        # 512 signals per core per launch

_F32 = mybir.dt.float32
_BF16 = mybir.dt.bfloat16
_FP8 = mybir.dt.float8e3
_NPBF16 = ml_dtypes.bfloat16
_NPFP8 = ml_dtypes.float8_e3m4
NWARM = 55                # PE p-state warmup matmuls (64 rows each)
FP8_SCALE = 2.5           # launch-2 input: F/sigma * FP8_SCALE fits e3m4 range

# stage-B kap2 iteration order: pairs (i, 32-i) so each kap2's -Bi view
# (which lives in block 32-kap2) is inside an already-streamed chunk
PI = []
for _i in range(1, 16):
    PI += [_i, 32 - _i]
PI += [16, 0]
SLOT = {k: s for s, k in enumerate(PI)}

# ---------------------------------------------------------------------------
# constants (host-side numpy)
# ---------------------------------------------------------------------------

_consts_cache = None


def _make_consts():
    """w1: [128, 128] f32 stage-A moving weights (launch scale folded later).
    w2: [128, 32, 2, 128] bf16 stage-B {Br, Bi} blocks in PI order."""
    global _consts_cache
    if _consts_cache is not None:
        return _consts_cache
    j2 = np.arange(32)
    W32 = np.exp(-2j * np.pi * np.outer(j2, j2) / 32)
    # M2[pl, j2, q, k2]: contribution of input plane pl to output plane q
    M2 = np.empty((2, 32, 2, 32), np.float64)
    M2[0, :, 0, :] = W32.real
    M2[1, :, 0, :] = -W32.imag
    M2[0, :, 1, :] = W32.imag
    M2[1, :, 1, :] = W32.real
    # W1[(u, pl, j2), (q, k2, u')] = delta_{u,u'} * M2
    W1 = np.zeros((2, 2, 32, 2, 32, 2), np.float64)
    for u in range(2):
        W1[u, :, :, :, :, u] = M2
    w1 = W1.reshape(128, 128).astype(np.float32)

    j1 = np.arange(128)
    kk = np.arange(4096)
    W4096 = np.exp(-2j * np.pi * np.outer(j1, kk) / 4096)  # [j1, k]
    w2 = np.empty((128, 32, 2, 128), np.float32)
    for s, kap2 in enumerate(PI):
        blk = W4096[:, kap2::32]           # [j1, k1] = B_kap2
        w2[:, s, 0] = blk.real
        w2[:, s, 1] = blk.imag
    w2 = w2.astype(_NPBF16)
    _consts_cache = (w1, w2)
    return _consts_cache


_tw_cache = None


def _global_twiddle():
    """exp(-2pi i k2g*j1g / N) as complex64 [NG, NG] (k2g rows)."""
    global _tw_cache
    if _tw_cache is None:
        k = np.arange(NG, dtype=np.float64)
        phase = np.outer(k, k) * (-2.0 * np.pi / N)
        _tw_cache = np.exp(1j * phase).astype(np.complex64)
    return _tw_cache


# ---------------------------------------------------------------------------
# marshalling (host)
# ---------------------------------------------------------------------------

def _marshal_in(Vre, Vim, npdt):
    """Vre/Vim: [4096 f][512 b] f32 planes -> in2 [128, 256, 128] npdt.

    in2[u*64 + pl*32 + j2, m, j1] = V_pl[j1 + 128*j2, 2*m + u]
    """
    V = np.stack([Vre, Vim])                    # [pl, f, b]
    V5 = V.reshape(2, 32, 128, 256, 2)          # pl, j2, j1, m, u
    out = V5.transpose(4, 0, 1, 3, 2).reshape(128, 256, 128)
    out = np.ascontiguousarray(out)
    if npdt is _NPFP8:
        out = np.clip(out, -15.5, 15.5)
    return out.astype(npdt)


def _unmarshal_out(O):
    """out2 [32, 128, 2, 512] bf16 (kap2, k1, pl, b) -> (Fre, Fim) planes
    [4096 k][512 b] f32, k = kap2 + 32*k1."""
    P = np.asarray(O).reshape(32, 128, 2, 512).transpose(2, 1, 0, 3)
    P = np.ascontiguousarray(P).astype(np.float32).reshape(2, 4096, 512)
    return P[0], P[1]


# ---------------------------------------------------------------------------
# device kernel (Bass/Tile); one module per input dtype
# ---------------------------------------------------------------------------

_nc_cache = {}

# input-DMA arrival ladder: m-chunk sizes (sum 256); fine early so the PE
# can start as soon as possible behind the stream
IN_SPLITS = [8, 8, 16, 16, 16, 32, 32, 32, 32, 32, 16, 8, 8]


def _build_nc(in_dt):
    if in_dt in _nc_cache:
        return _nc_cache[in_dt]
    split_b = False

    nc = bacc.Bacc(trn_type="TRN2")
    in_d = nc.dram_tensor("in2", [128, 256, 128], in_dt, kind="ExternalInput")
    w1_d = nc.dram_tensor("w1", [128, 128], _BF16, kind="ExternalInput")
    w2_d = nc.dram_tensor("w2", [128, 32, 2, 128], _BF16, kind="ExternalInput")
    out_d = nc.dram_tensor("out2", [32, 128, 2, 512], _BF16, kind="ExternalOutput")

    # PE p-state warmup seed: memset BEFORE the TileContext so the first
    # warmup matmul issues right after the preamble barrier; the clock ramp
    # then completes before the first data-gated matmul.
    wz = nc.alloc_sbuf_tensor("wz0", [128, 128], _BF16)
    nc.vector.memset(wz.ap(), 0.0)

    # weighted evacuation rotation: error-diffusion on per-copy engine time
    # (DVE + Activation only: GPSIMD cannot access PSUM on TRN2)
    _rates = {"A": 1.0 / 996.0, "D": 1.0 / 1192.0}
    _tot = sum(_rates.values())
    _shares = {k: v / _tot for k, v in _rates.items()}

    with tile.TileContext(nc) as tc:
        with (
            tc.tile_pool(name="consts", bufs=1) as cpool,
            tc.tile_pool(name="outp", bufs=6) as outpool,
            tc.tile_pool(name="ps", bufs=4, space="PSUM") as ps,
        ):
            w1_t = cpool.tile([128, 128], _BF16, tag="w1")
            nc.sync.dma_start(w1_t[:], w1_d.ap())

            in_t = cpool.tile([128, 256, 128], in_dt, tag="in")
            w2_t = cpool.tile([128, 32, 2, 128], _BF16, tag="w2")

            def in_chunks(splits, lo):
                for ch in splits:
                    nc.sync.dma_start(in_t[:, lo:lo + ch],
                                      in_d.ap()[:, lo:lo + ch])
                    lo += ch
                return lo

            def w2_chunks():
                # each 4-block chunk unblocks 4 kap2 iterations (PI order)
                for e in range(8):
                    nc.sync.dma_start(w2_t[:, 4 * e:4 * (e + 1)],
                                      w2_d.ap()[:, 4 * e:4 * (e + 1)])

            if split_b:
                # w2 mid-ladder: early enough for the interleaved stage-B
                # first half, late enough not to starve the PE's first groups
                lo = in_chunks(IN_SPLITS[:5], 0)
                w2_chunks()
                in_chunks(IN_SPLITS[5:], lo)
            else:
                # stage B begins only after all of stage A: w2 can trail
                in_chunks(IN_SPLITS, 0)
                w2_chunks()

            # PE p-state warmup: bridge the input-DMA latency with throwaway
            # matmuls so every real matmul runs at the full 2.4GHz clock.
            wb = ps.tile([128, 1024], _F32, tag="ps")
            for w in range(NWARM):
                nc.tensor.matmul(wb[:, :64], wz.ap(), wz.ap()[:, :64],
                                 start=True, stop=True)

            # preload the Activation copy table during the DMA wait
            warm_sb = cpool.tile([128, 1], _BF16, tag="actwarm")
            nc.scalar.copy(warm_sb[:], wz.ap()[:, :1])

            _err = dict.fromkeys(_rates, 0.0)

            def evac(out_ap, in_ap):
                for k in _err:
                    _err[k] += _shares[k]
                eng = max(_err, key=lambda k: _err[k])
                _err[eng] -= 1.0
                if eng == "D":
                    nc.vector.tensor_copy(out_ap, in_ap)
                else:
                    nc.scalar.copy(out_ap, in_ap)

            # ---- stage A: 256 data-stationary one-pass matmuls ----
            # T cols, flattened: (g, dm, q, kap2, u) — each bank lands
            # contiguously; stage-B slices are then b-ordered.
            T = cpool.tile([128, 64, 4, 2, 32, 2], _BF16, tag="T")
            Tf = T.rearrange("p a b c d e -> p (a b c d e)")

            def a_pair(gp):
                # two 4-m groups share one 2-bank PSUM tile: a single
                # spanning evacuation halves the per-instruction PSUM-access
                # overhead on the (binding) evac engines
                bank = ps.tile([128, 1024], _F32, tag="ps")
                for dm in range(8):
                    m = 8 * gp + dm
                    nc.tensor.matmul(bank[:, 128 * dm:128 * (dm + 1)],
                                     in_t[:, m], w1_t[:],
                                     start=True, stop=True)
                evac(Tf[:, 1024 * gp:1024 * (gp + 1)], bank[:])

            wn_t = cpool.tile([128, 32, 128], _BF16, tag="wn")
            _negated = set()

            def negate_chunk(e):
                # -Bi for blocks [4e, 4e+4): DVE/ACT alternate; emitted just
                # before first use so they slot behind the stage-A evacs
                if e in _negated:
                    return
                _negated.add(e)
                dst = wn_t[:, 4 * e:4 * (e + 1)]
                srcn = w2_t[:, 4 * e:4 * (e + 1), 1]
                if e % 2 == 0:
                    nc.vector.tensor_scalar_mul(dst, srcn, -1.0)
                else:
                    nc.scalar.mul(dst, srcn, -1.0)

            def b_weights(s, kap2):
                br = w2_t[:, s, 0]
                bi = w2_t[:, s, 1]
                bni = wn_t[:, s]
                return br, bi, bni

            def b_half(s, kap2, h, wide):
                gs = slice(0, 64) if wide else slice(32 * h, 32 * h + 32)
                nb = 512 if wide else 256
                tr = T[:, gs, :, 0, kap2, :]
                ti = T[:, gs, :, 1, kap2, :]
                br, bi, bni = b_weights(s, kap2)
                yrz = ps.tile([128, 2 * nb], _F32, tag="ps")
                yr = yrz[:, 0:nb]
                yi = yrz[:, nb:2 * nb]
                nc.tensor.matmul(yr, br, tr, start=True, stop=False)
                nc.tensor.matmul(yr, bni, ti, start=False, stop=True)
                nc.tensor.matmul(yi, bi, tr, start=True, stop=False)
                nc.tensor.matmul(yi, br, ti, start=False, stop=True)
                ot = outpool.tile([128, 2, nb], _BF16, tag="out")
                otf = ot.rearrange("p a b -> p (a b)")
                if wide and s == len(PI) - 1:
                    # final-kap2 latency chain: evacuate the planes on both
                    # engines in parallel and ship each as its own DMA on a
                    # different queue so gens and transfers overlap
                    nc.vector.tensor_copy(ot[:, 0], yr)
                    nc.gpsimd.dma_start(out_d[kap2][:, 0], ot[:, 0])
                    nc.scalar.copy(ot[:, 1], yi)
                    nc.sync.dma_start(out_d[kap2][:, 1], ot[:, 1])
                    return
                # (re | im) blocks of yrz match ot's (pl, b) layout: one
                # spanning evacuation covers the whole kap2 output
                evac(otf[:], yrz[:])
                if wide:
                    nc.sync.dma_start(out_d[kap2], ot[:])
                elif h == 0:
                    # first-half outs ride SWDGE (Pool) — keeps the 32 extra
                    # desc-gens off the serialized HWDGE device
                    nc.gpsimd.dma_start(out_d[kap2][:, :, 0:256], ot[:])
                else:
                    nc.sync.dma_start(out_d[kap2][:, :, 256:512], ot[:])

            if not split_b:
                for gp in range(32):
                    a_pair(gp)
                for s, kap2 in enumerate(PI):
                    negate_chunk(s // 4)
                    b_half(s, kap2, 0, True)
            else:
                # interleave: stage-B's first batch-half rides along as soon
                # as the first half of T exists, filling the PE while stage A
                # is evacuation-paced
                for gp in range(16):
                    a_pair(gp)
                for s, kap2 in enumerate(PI):
                    negate_chunk(s // 4)
                    b_half(s, kap2, 0, False)
                    if s < 16:
                        a_pair(16 + s)
                for s, kap2 in enumerate(PI):
                    b_half(s, kap2, 1, False)

    nc.finalize()
    _nc_cache[in_dt] = nc
    return nc


# ---------------------------------------------------------------------------
# launch helper
# ---------------------------------------------------------------------------

_last_exec_ns = None


def last_exec_time_ns():
    """Sum of HW exec times (ns) of the launches in the last kernel() call,
    when KERNEL_TRACE=1 was set and NTFF profiling is available. None otherwise."""
    return _last_exec_ns


def predicted_exec_time_ns():
    """Cost-model (TimelineSim) predicted HW exec time for both launches, ns."""
    from concourse.timeline_sim import TimelineSim
    total = 0
    for dt_ in (_BF16, _FP8):
        total += int(TimelineSim(_build_nc(dt_)).simulate())
    return total


def _run_launch(cols_re, cols_im, in_dt, npdt, w1_scale):
    """cols_re/cols_im: list of 8 planes [4096 f][512 b] f32.
    Returns list of 8 (Fre, Fim) planes [4096 k][512 b]."""
    global _last_exec_ns
    import os
    nc = _build_nc(in_dt)
    w1, w2 = _make_consts()
    w1s = (w1 * w1_scale).astype(_NPBF16)
    in_maps = []
    for c in range(NCORES):
        in_maps.append({
            "in2": _marshal_in(cols_re[c], cols_im[c], npdt),
            "w1": w1s, "w2": w2,
        })
    trace = bool(os.environ.get("KERNEL_TRACE"))
    try:
        res = run_bass_kernel_spmd(nc, in_maps, core_ids=list(range(NCORES)),
                                   trace=trace)
    except ModuleNotFoundError:
        res = run_bass_kernel_spmd(nc, in_maps, core_ids=list(range(NCORES)))
    if trace and getattr(res, "exec_time_ns", None) is not None:
        _last_exec_ns = (_last_exec_ns or 0) + res.exec_time_ns
    return [_unmarshal_out(res.results[c]["out2"]) for c in range(NCORES)]


# ---------------------------------------------------------------------------
# public entry point
# ---------------------------------------------------------------------------

def kernel(x: np.ndarray) -> np.ndarray:
    """x: [N, 2] float32 (re, im). Returns FFT(x) as [N, 2] float32."""
    global _last_exec_ns
    _last_exec_ns = None
    x = np.asarray(x)
    Are = np.ascontiguousarray(x[:, 0].reshape(NG, NG))  # [j2g][j1g]
    Aim = np.ascontiguousarray(x[:, 1].reshape(NG, NG))

    # launch 1 (bf16 input): FFT over rows (j2g) for each column j1g
    cols_re = [np.ascontiguousarray(Are[:, BPC * c:BPC * (c + 1)]) for c in range(NCORES)]
    cols_im = [np.ascontiguousarray(Aim[:, BPC * c:BPC * (c + 1)]) for c in range(NCORES)]
    l1 = _run_launch(cols_re, cols_im, _BF16, _NPBF16, 1.0)

    # host: assemble F [k2g][j1g], twiddle, transpose-exchange
    F = np.empty((NG, NG), np.complex64)
    for c in range(NCORES):
        fre, fim = l1[c]
        F[:, BPC * c:BPC * (c + 1)] = fre + 1j * fim
    F *= _global_twiddle()

    # launch 2 (fp8 input): FFT over j1g for each row k2g; rescale to
    # FP8_SCALE/sigma for e3m4, undo via the stage-A weights.
    sigma = float(np.sqrt(np.mean(F.real.astype(np.float64) ** 2
                                  + F.imag.astype(np.float64) ** 2) / 2.0))
    s2 = FP8_SCALE / sigma
    cols_re2 = []
    cols_im2 = []
    for d in range(NCORES):
        block = F[BPC * d:BPC * (d + 1), :].T      # [j1g][k2g-local]
        cols_re2.append(np.ascontiguousarray(block.real) * np.float32(s2))
        cols_im2.append(np.ascontiguousarray(block.imag) * np.float32(s2))
    l2 = _run_launch(cols_re2, cols_im2, _FP8, _NPFP8, 1.0 / s2)

    # assemble Xmat [k1g][k2g]; out flat index k = 4096*k1g + k2g
    out = np.empty((NG, NG, 2), np.float32)
    for d in range(NCORES):
        rre, rim = l2[d]
        out[:, BPC * d:BPC * (d + 1), 0] = rre
        out[:, BPC * d:BPC * (d + 1), 1] = rim
    return out.reshape(N, 2)


# revision 29
# speedup vs baseline: 1.0101x; 1.0101x over previous
"""Distributed FFT (N = 2^24 complex points) on 8 Trainium2 NeuronCores.

Four-step (Cooley-Tukey) decomposition N = 4096 x 4096:
  launch 1: per global column j1g, FFT_4096 over j2g      (batch parallel over j1g)
  host:     global twiddle wN^{j1g*k2g} + transpose exchange
  launch 2: per global row k2g, FFT_4096 over j1g         (batch parallel over k2g)

Each local FFT_4096 = radix-32 stage A fused with its inter-stage transpose,
then a radix-128 stage B with per-kap2 twiddle-folded weights.

Stage A is a ONE-PASS complex matmul: the contraction axis K = 128 packs
(u-batch-bit x plane x j2) and the moving weight W1 is the 2x2 real
representation of W32, block-diagonal over u. Each data-stationary matmul
(lhsT = a [128, 128] input slice covering 2 signals) emits both output planes
in a single 128-column stream, halving stage-A PE time vs the 2-pass form.
Stage B stays 2-pass (the plane axis leaves the partition dim after stage A -
a lane-locked-evacuation parity constraint makes consecutive 1-pass complex
stages impossible), giving 98304 PE streaming cycles/launch vs 131072 before.

Stage-B weights: B_kap2[j1,k1] = W4096[j1, kap2+32*k1], so {Br, Bi} ship as
one table w2 (2.1MB, kap2-blocked) and -Bi is generated on device by cheap
DVE/ACT negates as each chunk lands (the BIR verifier forbids negative-stride
stationaries, so the reversed-view trick is out). That kills the redundant
third matrix of the old 3.1MB layout and lets w2 stream AFTER the input
ladder, in 8 chunks that each unblock 4 kap2 iterations.

PSUM evacuations rotate over DVE / Activation (the only PSUM-capable
engines; error-diffusion weighted by per-copy time) so neither engine's copy
throughput (~1 col/cycle at 0.96-1.2 GHz) gates the PE (2.4 GHz). PSUM tiles
span TWO banks ([128, 1024]) so one evacuation drains two stage-A groups or
a whole kap2's (re | im) pair, halving the per-instruction PSUM-access
overhead on the binding evac engines; the 8 banks rotate as 4 such tiles.

DMA: all queues serialize at 360 GB/s in the cost model, so bytes are the
floor. Launch-2's input (the twiddled intermediate, host-rescaled) ships as
float8_e3m4 (1.33% rms quantization, measured; total error ~1.4% vs the 2e-2
gate), halving that launch's input traffic. Launch-1 input and all weights
stay bf16. Outputs stay bf16.

Layouts (per launch, per core, batch 512 signals of 4096 points):
  f = j1 + 128*j2; k = kap2 + 32*kap1; b = 2*m + u (m in [0,256)).
  in  [p, m, j1]        p = u*64 + pl*32 + j2
  T   [j1, g, dm, q, kap2, u]   (g, dm) = m split 64x4; stage-B slice is
                                 b-ordered: psum cols (g, dm, u) = b
  out [kap2, k1, pl, b]
"""
import numpy as np
import ml_dtypes

import concourse.mybir as mybir
import concourse.tile as tile
from concourse import bacc
from concourse.bass_utils import run_bass_kernel_spmd

NG = 4096                 # global matrix dimension; N = NG*NG
N = NG * NG
NCORES = 8
BPC = NG // NCORES        # 512 signals per core per launch

_F32 = mybir.dt.float32
_BF16 = mybir.dt.bfloat16
_FP8 = mybir.dt.float8e3
_NPBF16 = ml_dtypes.bfloat16
_NPFP8 = ml_dtypes.float8_e3m4
NWARM = 55                # PE p-state warmup matmuls (64 rows each)
FP8_SCALE = 2.5           # launch-2 input: F/sigma * FP8_SCALE fits e3m4 range

# stage-B kap2 iteration order: pairs (i, 32-i) so each kap2's -Bi view
# (which lives in block 32-kap2) is inside an already-streamed chunk
PI = []
for _i in range(1, 16):
    PI += [_i, 32 - _i]
PI += [16, 0]
SLOT = {k: s for s, k in enumerate(PI)}

# ---------------------------------------------------------------------------
# constants (host-side numpy)
# ---------------------------------------------------------------------------

_consts_cache = None


def _make_consts():
    """w1: [128, 128] f32 stage-A moving weights (launch scale folded later).
    w2: [128, 32, 2, 128] bf16 stage-B {Br, Bi} blocks in PI order."""
    global _consts_cache
    if _consts_cache is not None:
        return _consts_cache
    j2 = np.arange(32)
    W32 = np.exp(-2j * np.pi * np.outer(j2, j2) / 32)
    # M2[pl, j2, q, k2]: contribution of input plane pl to output plane q
    M2 = np.empty((2, 32, 2, 32), np.float64)
    M2[0, :, 0, :] = W32.real
    M2[1, :, 0, :] = -W32.imag
    M2[0, :, 1, :] = W32.imag
    M2[1, :, 1, :] = W32.real
    # W1[(u, pl, j2), (q, k2, u')] = delta_{u,u'} * M2
    W1 = np.zeros((2, 2, 32, 2, 32, 2), np.float64)
    for u in range(2):
        W1[u, :, :, :, :, u] = M2
    w1 = W1.reshape(128, 128).astype(np.float32)

    j1 = np.arange(128)
    kk = np.arange(4096)
    W4096 = np.exp(-2j * np.pi * np.outer(j1, kk) / 4096)  # [j1, k]
    w2 = np.empty((128, 32, 2, 128), np.float32)
    for s, kap2 in enumerate(PI):
        blk = W4096[:, kap2::32]           # [j1, k1] = B_kap2
        w2[:, s, 0] = blk.real
        w2[:, s, 1] = blk.imag
    w2 = w2.astype(_NPBF16)
    _consts_cache = (w1, w2)
    return _consts_cache


_tw_cache = None


def _global_twiddle():
    """exp(-2pi i k2g*j1g / N) as complex64 [NG, NG] (k2g rows)."""
    global _tw_cache
    if _tw_cache is None:
        k = np.arange(NG, dtype=np.float64)
        phase = np.outer(k, k) * (-2.0 * np.pi / N)
        _tw_cache = np.exp(1j * phase).astype(np.complex64)
    return _tw_cache


# ---------------------------------------------------------------------------
# marshalling (host)
# ---------------------------------------------------------------------------

def _marshal_in(Vre, Vim, npdt):
    """Vre/Vim: [4096 f][512 b] f32 planes -> in2 [128, 256, 128] npdt.

    in2[u*64 + pl*32 + j2, m, j1] = V_pl[j1 + 128*j2, 2*m + u]
    """
    V = np.stack([Vre, Vim])                    # [pl, f, b]
    V5 = V.reshape(2, 32, 128, 256, 2)          # pl, j2, j1, m, u
    out = V5.transpose(4, 0, 1, 3, 2).reshape(128, 256, 128)
    out = np.ascontiguousarray(out)
    if npdt is _NPFP8:
        out = np.clip(out, -15.5, 15.5)
    return out.astype(npdt)


def _unmarshal_out(O):
    """out2 [32, 128, 2, 512] bf16 (kap2, k1, pl, b) -> (Fre, Fim) planes
    [4096 k][512 b] f32, k = kap2 + 32*k1."""
    P = np.asarray(O).reshape(32, 128, 2, 512).transpose(2, 1, 0, 3)
    P = np.ascontiguousarray(P).astype(np.float32).reshape(2, 4096, 512)
    return P[0], P[1]


# ---------------------------------------------------------------------------
# device kernel (Bass/Tile); one module per input dtype
# ---------------------------------------------------------------------------

_nc_cache = {}

# input-DMA arrival ladder: m-chunk sizes (sum 256); fine early so the PE
# can start as soon as possible behind the stream
IN_SPLITS = [8, 8, 16, 16, 16, 32, 32, 32, 32, 32, 16, 8, 8]


def _build_nc(in_dt):
    if in_dt in _nc_cache:
        return _nc_cache[in_dt]
    split_b = False

    nc = bacc.Bacc(trn_type="TRN2")
    in_d = nc.dram_tensor("in2", [128, 256, 128], in_dt, kind="ExternalInput")
    w1_d = nc.dram_tensor("w1", [128, 128], _BF16, kind="ExternalInput")
    w2_d = nc.dram_tensor("w2", [128, 32, 2, 128], _BF16, kind="ExternalInput")
    out_d = nc.dram_tensor("out2", [32, 128, 2, 512], _BF16, kind="ExternalOutput")

    # PE p-state warmup seed: memset BEFORE the TileContext so the first
    # warmup matmul issues right after the preamble barrier; the clock ramp
    # then completes before the first data-gated matmul.
    wz = nc.alloc_sbuf_tensor("wz0", [128, 128], _BF16)
    nc.vector.memset(wz.ap(), 0.0)

    # weighted evacuation rotation: error-diffusion on per-copy engine time
    # (DVE + Activation only: GPSIMD cannot access PSUM on TRN2)
    _rates = {"A": 1.0 / 996.0, "D": 1.0 / 1192.0}
    _tot = sum(_rates.values())
    _shares = {k: v / _tot for k, v in _rates.items()}

    with tile.TileContext(nc) as tc:
        with (
            tc.tile_pool(name="consts", bufs=1) as cpool,
            tc.tile_pool(name="outp", bufs=6) as outpool,
            tc.tile_pool(name="ps", bufs=4, space="PSUM") as ps,
        ):
            w1_t = cpool.tile([128, 128], _BF16, tag="w1")
            nc.sync.dma_start(w1_t[:], w1_d.ap())

            in_t = cpool.tile([128, 256, 128], in_dt, tag="in")
            w2_t = cpool.tile([128, 32, 2, 128], _BF16, tag="w2")

            def in_chunks(splits, lo):
                for ch in splits:
                    nc.sync.dma_start(in_t[:, lo:lo + ch],
                                      in_d.ap()[:, lo:lo + ch])
                    lo += ch
                return lo

            def w2_chunks():
                # each 4-block chunk unblocks 4 kap2 iterations (PI order)
                for e in range(8):
                    nc.sync.dma_start(w2_t[:, 4 * e:4 * (e + 1)],
                                      w2_d.ap()[:, 4 * e:4 * (e + 1)])

            if split_b:
                # w2 mid-ladder: early enough for the interleaved stage-B
                # first half, late enough not to starve the PE's first groups
                lo = in_chunks(IN_SPLITS[:5], 0)
                w2_chunks()
                in_chunks(IN_SPLITS[5:], lo)
            else:
                # stage B begins only after all of stage A: w2 can trail
                in_chunks(IN_SPLITS, 0)
                w2_chunks()

            # PE p-state warmup: bridge the input-DMA latency with throwaway
            # matmuls so every real matmul runs at the full 2.4GHz clock.
            wb = ps.tile([128, 1024], _F32, tag="ps")
            for w in range(NWARM):
                nc.tensor.matmul(wb[:, :64], wz.ap(), wz.ap()[:, :64],
                                 start=True, stop=True)

            # preload the Activation copy table during the DMA wait
            warm_sb = cpool.tile([128, 1], _BF16, tag="actwarm")
            nc.scalar.copy(warm_sb[:], wz.ap()[:, :1])

            _err = dict.fromkeys(_rates, 0.0)

            def evac(out_ap, in_ap):
                for k in _err:
                    _err[k] += _shares[k]
                eng = max(_err, key=lambda k: _err[k])
                _err[eng] -= 1.0
                if eng == "D":
                    nc.vector.tensor_copy(out_ap, in_ap)
                else:
                    nc.scalar.copy(out_ap, in_ap)

            # ---- stage A: 256 data-stationary one-pass matmuls ----
            # T cols, flattened: (g, dm, q, kap2, u) — each bank lands
            # contiguously; stage-B slices are then b-ordered.
            T = cpool.tile([128, 64, 4, 2, 32, 2], _BF16, tag="T")
            Tf = T.rearrange("p a b c d e -> p (a b c d e)")

            def a_pair(gp):
                # two 4-m groups share one 2-bank PSUM tile: a single
                # spanning evacuation halves the per-instruction PSUM-access
                # overhead on the (binding) evac engines. The final pair
                # instead uses two independent tiles whose evacs rotate onto
                # both engines, halving the last-evac latency gating stage B.
                if gp == 31:
                    for half in range(2):
                        g = 2 * gp + half
                        bank = ps.tile([128, 512], _F32, tag="ps")
                        for dm in range(4):
                            m = 4 * g + dm
                            nc.tensor.matmul(
                                bank[:, 128 * dm:128 * (dm + 1)],
                                in_t[:, m], w1_t[:], start=True, stop=True)
                        evac(Tf[:, 512 * g:512 * (g + 1)], bank[:])
                    return
                bank = ps.tile([128, 1024], _F32, tag="ps")
                for dm in range(8):
                    m = 8 * gp + dm
                    nc.tensor.matmul(bank[:, 128 * dm:128 * (dm + 1)],
                                     in_t[:, m], w1_t[:],
                                     start=True, stop=True)
                evac(Tf[:, 1024 * gp:1024 * (gp + 1)], bank[:])

            wn_t = cpool.tile([128, 32, 128], _BF16, tag="wn")
            _negated = set()

            def negate_chunk(e):
                # -Bi for blocks [4e, 4e+4): DVE/ACT alternate; emitted just
                # before first use so they slot behind the stage-A evacs
                if e in _negated:
                    return
                _negated.add(e)
                dst = wn_t[:, 4 * e:4 * (e + 1)]
                srcn = w2_t[:, 4 * e:4 * (e + 1), 1]
                if e % 2 == 0:
                    nc.vector.tensor_scalar_mul(dst, srcn, -1.0)
                else:
                    nc.scalar.mul(dst, srcn, -1.0)

            def b_weights(s, kap2):
                br = w2_t[:, s, 0]
                bi = w2_t[:, s, 1]
                bni = wn_t[:, s]
                return br, bi, bni

            def b_half(s, kap2, h, wide):
                gs = slice(0, 64) if wide else slice(32 * h, 32 * h + 32)
                nb = 512 if wide else 256
                tr = T[:, gs, :, 0, kap2, :]
                ti = T[:, gs, :, 1, kap2, :]
                br, bi, bni = b_weights(s, kap2)
                yrz = ps.tile([128, 2 * nb], _F32, tag="ps")
                yr = yrz[:, 0:nb]
                yi = yrz[:, nb:2 * nb]
                nc.tensor.matmul(yr, br, tr, start=True, stop=False)
                nc.tensor.matmul(yr, bni, ti, start=False, stop=True)
                nc.tensor.matmul(yi, bi, tr, start=True, stop=False)
                nc.tensor.matmul(yi, br, ti, start=False, stop=True)
                ot = outpool.tile([128, 2, nb], _BF16, tag="out")
                otf = ot.rearrange("p a b -> p (a b)")
                if wide and s == len(PI) - 1:
                    # final-kap2 latency chain: evacuate the planes on both
                    # engines in parallel and ship each as its own DMA on a
                    # different queue so gens and transfers overlap
                    nc.vector.tensor_copy(ot[:, 0], yr)
                    nc.gpsimd.dma_start(out_d[kap2][:, 0], ot[:, 0])
                    nc.scalar.copy(ot[:, 1], yi)
                    nc.sync.dma_start(out_d[kap2][:, 1], ot[:, 1])
                    return
                # (re | im) blocks of yrz match ot's (pl, b) layout: one
                # spanning evacuation covers the whole kap2 output
                evac(otf[:], yrz[:])
                if wide:
                    nc.sync.dma_start(out_d[kap2], ot[:])
                elif h == 0:
                    # first-half outs ride SWDGE (Pool) — keeps the 32 extra
                    # desc-gens off the serialized HWDGE device
                    nc.gpsimd.dma_start(out_d[kap2][:, :, 0:256], ot[:])
                else:
                    nc.sync.dma_start(out_d[kap2][:, :, 256:512], ot[:])

            if not split_b:
                for gp in range(32):
                    a_pair(gp)
                for s, kap2 in enumerate(PI):
                    negate_chunk(s // 4)
                    b_half(s, kap2, 0, True)
            else:
                # interleave: stage-B's first batch-half rides along as soon
                # as the first half of T exists, filling the PE while stage A
                # is evacuation-paced
                for gp in range(16):
                    a_pair(gp)
                for s, kap2 in enumerate(PI):
                    negate_chunk(s // 4)
                    b_half(s, kap2, 0, False)
                    if s < 16:
                        a_pair(16 + s)
                for s, kap2 in enumerate(PI):
                    b_half(s, kap2, 1, False)

    nc.finalize()
    _nc_cache[in_dt] = nc
    return nc


# ---------------------------------------------------------------------------
# launch helper
# ---------------------------------------------------------------------------

_last_exec_ns = None


def last_exec_time_ns():
    """Sum of HW exec times (ns) of the launches in the last kernel() call,
    when KERNEL_TRACE=1 was set and NTFF profiling is available. None otherwise."""
    return _last_exec_ns


def predicted_exec_time_ns():
    """Cost-model (TimelineSim) predicted HW exec time for both launches, ns."""
    from concourse.timeline_sim import TimelineSim
    total = 0
    for dt_ in (_BF16, _FP8):
        total += int(TimelineSim(_build_nc(dt_)).simulate())
    return total


def _run_launch(cols_re, cols_im, in_dt, npdt, w1_scale):
    """cols_re/cols_im: list of 8 planes [4096 f][512 b] f32.
    Returns list of 8 (Fre, Fim) planes [4096 k][512 b]."""
    global _last_exec_ns
    import os
    nc = _build_nc(in_dt)
    w1, w2 = _make_consts()
    w1s = (w1 * w1_scale).astype(_NPBF16)
    in_maps = []
    for c in range(NCORES):
        in_maps.append({
            "in2": _marshal_in(cols_re[c], cols_im[c], npdt),
            "w1": w1s, "w2": w2,
        })
    trace = bool(os.environ.get("KERNEL_TRACE"))
    try:
        res = run_bass_kernel_spmd(nc, in_maps, core_ids=list(range(NCORES)),
                                   trace=trace)
    except ModuleNotFoundError:
        res = run_bass_kernel_spmd(nc, in_maps, core_ids=list(range(NCORES)))
    if trace and getattr(res, "exec_time_ns", None) is not None:
        _last_exec_ns = (_last_exec_ns or 0) + res.exec_time_ns
    return [_unmarshal_out(res.results[c]["out2"]) for c in range(NCORES)]


# ---------------------------------------------------------------------------
# public entry point
# ---------------------------------------------------------------------------

def kernel(x: np.ndarray) -> np.ndarray:
    """x: [N, 2] float32 (re, im). Returns FFT(x) as [N, 2] float32."""
    global _last_exec_ns
    _last_exec_ns = None
    x = np.asarray(x)
    Are = np.ascontiguousarray(x[:, 0].reshape(NG, NG))  # [j2g][j1g]
    Aim = np.ascontiguousarray(x[:, 1].reshape(NG, NG))

    # launch 1 (bf16 input): FFT over rows (j2g) for each column j1g
    cols_re = [np.ascontiguousarray(Are[:, BPC * c:BPC * (c + 1)]) for c in range(NCORES)]
    cols_im = [np.ascontiguousarray(Aim[:, BPC * c:BPC * (c + 1)]) for c in range(NCORES)]
    l1 = _run_launch(cols_re, cols_im, _BF16, _NPBF16, 1.0)

    # host: assemble F [k2g][j1g], twiddle, transpose-exchange
    F = np.empty((NG, NG), np.complex64)
    for c in range(NCORES):
        fre, fim = l1[c]
        F[:, BPC * c:BPC * (c + 1)] = fre + 1j * fim
    F *= _global_twiddle()

    # launch 2 (fp8 input): FFT over j1g for each row k2g; rescale to
    # FP8_SCALE/sigma for e3m4, undo via the stage-A weights.
    sigma = float(np.sqrt(np.mean(F.real.astype(np.float64) ** 2
                                  + F.imag.astype(np.float64) ** 2) / 2.0))
    s2 = FP8_SCALE / sigma
    cols_re2 = []
    cols_im2 = []
    for d in range(NCORES):
        block = F[BPC * d:BPC * (d + 1), :].T      # [j1g][k2g-local]
        cols_re2.append(np.ascontiguousarray(block.real) * np.float32(s2))
        cols_im2.append(np.ascontiguousarray(block.imag) * np.float32(s2))
    l2 = _run_launch(cols_re2, cols_im2, _FP8, _NPFP8, 1.0 / s2)

    # assemble Xmat [k1g][k2g]; out flat index k = 4096*k1g + k2g
    out = np.empty((NG, NG, 2), np.float32)
    for d in range(NCORES):
        rre, rim = l2[d]
        out[:, BPC * d:BPC * (d + 1), 0] = rre
        out[:, BPC * d:BPC * (d + 1), 1] = rim
    return out.reshape(N, 2)


# revision 32
# speedup vs baseline: 1.0189x; 1.0087x over previous
"""Distributed FFT (N = 2^24 complex points) on 8 Trainium2 NeuronCores.

Four-step (Cooley-Tukey) decomposition N = 4096 x 4096:
  launch 1: per global column j1g, FFT_4096 over j2g      (batch parallel over j1g)
  host:     global twiddle wN^{j1g*k2g} + transpose exchange
  launch 2: per global row k2g, FFT_4096 over j1g         (batch parallel over k2g)

Each local FFT_4096 = radix-32 stage A fused with its inter-stage transpose,
then a radix-128 stage B with per-kap2 twiddle-folded weights.

Stage A is a ONE-PASS complex matmul: the contraction axis K = 128 packs
(u-batch-bit x plane x j2) and the moving weight W1 is the 2x2 real
representation of W32, block-diagonal over u. Each data-stationary matmul
(lhsT = a [128, 128] input slice covering 2 signals) emits both output planes
in a single 128-column stream, halving stage-A PE time vs the 2-pass form.
Stage B stays 2-pass (the plane axis leaves the partition dim after stage A -
a lane-locked-evacuation parity constraint makes consecutive 1-pass complex
stages impossible), giving 98304 PE streaming cycles/launch vs 131072 before.

Stage-B weights: B_kap2[j1,k1] = W4096[j1, kap2+32*k1], so {Br, Bi} ship as
one table w2 (2.1MB, kap2-blocked) and -Bi is generated on device by cheap
DVE/ACT negates as each chunk lands (the BIR verifier forbids negative-stride
stationaries, so the reversed-view trick is out). That kills the redundant
third matrix of the old 3.1MB layout and lets w2 stream AFTER the input
ladder, in 8 chunks that each unblock 4 kap2 iterations.

PSUM evacuations rotate over DVE / Activation (the only PSUM-capable
engines; error-diffusion weighted by per-copy time) so neither engine's copy
throughput (~1 col/cycle at 0.96-1.2 GHz) gates the PE (2.4 GHz). PSUM tiles
span TWO banks ([128, 1024]) so one evacuation drains two stage-A groups or
a whole kap2's (re | im) pair, halving the per-instruction PSUM-access
overhead on the binding evac engines; the 8 banks rotate as 4 such tiles.

DMA: all queues serialize at 360 GB/s in the cost model, so bytes are the
floor. Launch-2's input (the twiddled intermediate, host-rescaled) ships as
float8_e3m4 (1.33% rms quantization, measured; total error ~1.4% vs the 2e-2
gate), halving that launch's input traffic. Launch-1 input and all weights
stay bf16. Outputs stay bf16.

Layouts (per launch, per core, batch 512 signals of 4096 points):
  f = j1 + 128*j2; k = kap2 + 32*kap1; b = 2*m + u (m in [0,256)).
  in  [p, m, j1]        p = u*64 + pl*32 + j2
  T   [j1, g, dm, q, kap2, u]   (g, dm) = m split 64x4; stage-B slice is
                                 b-ordered: psum cols (g, dm, u) = b
  out [kap2, k1, pl, b]
"""
import numpy as np
import ml_dtypes

import concourse.mybir as mybir
import concourse.tile as tile
from concourse import bacc
from concourse.bass_utils import run_bass_kernel_spmd

NG = 4096                 # global matrix dimension; N = NG*NG
N = NG * NG
NCORES = 8
BPC = NG // NCORES        # 512 signals per core per launch

_F32 = mybir.dt.float32
_BF16 = mybir.dt.bfloat16
_FP8 = mybir.dt.float8e3
_NPBF16 = ml_dtypes.bfloat16
_NPFP8 = ml_dtypes.float8_e3m4
NWARM = 42                # PE p-state warmup matmuls (64 rows each)
FP8_SCALE = 2.5           # launch-2 input: F/sigma * FP8_SCALE fits e3m4 range

# stage-B kap2 iteration order: pairs (i, 32-i) so each kap2's -Bi view
# (which lives in block 32-kap2) is inside an already-streamed chunk
PI = []
for _i in range(1, 16):
    PI += [_i, 32 - _i]
PI += [16, 0]
SLOT = {k: s for s, k in enumerate(PI)}

# ---------------------------------------------------------------------------
# constants (host-side numpy)
# ---------------------------------------------------------------------------

_consts_cache = None


def _make_consts():
    """w1: [128, 128] f32 stage-A moving weights (launch scale folded later).
    w2: [128, 32, 2, 128] bf16 stage-B {Br, Bi} blocks in PI order."""
    global _consts_cache
    if _consts_cache is not None:
        return _consts_cache
    j2 = np.arange(32)
    W32 = np.exp(-2j * np.pi * np.outer(j2, j2) / 32)
    # M2[pl, j2, q, k2]: contribution of input plane pl to output plane q
    M2 = np.empty((2, 32, 2, 32), np.float64)
    M2[0, :, 0, :] = W32.real
    M2[1, :, 0, :] = -W32.imag
    M2[0, :, 1, :] = W32.imag
    M2[1, :, 1, :] = W32.real
    # W1[(u, pl, j2), (q, k2, u')] = delta_{u,u'} * M2
    W1 = np.zeros((2, 2, 32, 2, 32, 2), np.float64)
    for u in range(2):
        W1[u, :, :, :, :, u] = M2
    w1 = W1.reshape(128, 128).astype(np.float32)

    j1 = np.arange(128)
    kk = np.arange(4096)
    W4096 = np.exp(-2j * np.pi * np.outer(j1, kk) / 4096)  # [j1, k]
    w2 = np.empty((128, 32, 2, 128), np.float32)
    for s, kap2 in enumerate(PI):
        blk = W4096[:, kap2::32]           # [j1, k1] = B_kap2
        w2[:, s, 0] = blk.real
        w2[:, s, 1] = blk.imag
    w2 = w2.astype(_NPBF16)
    _consts_cache = (w1, w2)
    return _consts_cache


_tw_cache = None


def _global_twiddle():
    """exp(-2pi i k2g*j1g / N) as complex64 [NG, NG] (k2g rows)."""
    global _tw_cache
    if _tw_cache is None:
        k = np.arange(NG, dtype=np.float64)
        phase = np.outer(k, k) * (-2.0 * np.pi / N)
        _tw_cache = np.exp(1j * phase).astype(np.complex64)
    return _tw_cache


# ---------------------------------------------------------------------------
# marshalling (host)
# ---------------------------------------------------------------------------

def _marshal_in(Vre, Vim, npdt):
    """Vre/Vim: [4096 f][512 b] f32 planes -> in2 [128, 256, 128] npdt.

    in2[u*64 + pl*32 + j2, m, j1] = V_pl[j1 + 128*j2, 2*m + u]
    """
    V = np.stack([Vre, Vim])                    # [pl, f, b]
    V5 = V.reshape(2, 32, 128, 256, 2)          # pl, j2, j1, m, u
    out = V5.transpose(4, 0, 1, 3, 2).reshape(128, 256, 128)
    out = np.ascontiguousarray(out)
    if npdt is _NPFP8:
        out = np.clip(out, -15.5, 15.5)
    return out.astype(npdt)


def _unmarshal_out(O):
    """out2 [32, 128, 2, 512] bf16 (kap2, k1, pl, b) -> (Fre, Fim) planes
    [4096 k][512 b] f32, k = kap2 + 32*k1."""
    P = np.asarray(O).reshape(32, 128, 2, 512).transpose(2, 1, 0, 3)
    P = np.ascontiguousarray(P).astype(np.float32).reshape(2, 4096, 512)
    return P[0], P[1]


# ---------------------------------------------------------------------------
# device kernel (Bass/Tile); one module per input dtype
# ---------------------------------------------------------------------------

_nc_cache = {}

# input-DMA arrival ladder: m-chunk sizes (sum 256); fine early so the PE
# can start as soon as possible behind the stream
IN_SPLITS = [8, 8, 16, 16, 16, 32, 32, 32, 32, 32, 16, 8, 8]


def _build_nc(in_dt):
    if in_dt in _nc_cache:
        return _nc_cache[in_dt]
    split_b = False

    nc = bacc.Bacc(trn_type="TRN2")
    in_d = nc.dram_tensor("in2", [128, 256, 128], in_dt, kind="ExternalInput")
    w1_d = nc.dram_tensor("w1", [128, 128], _BF16, kind="ExternalInput")
    w2_d = nc.dram_tensor("w2", [128, 32, 2, 128], _BF16, kind="ExternalInput")
    out_d = nc.dram_tensor("out2", [32, 128, 2, 512], _BF16, kind="ExternalOutput")

    # PE p-state warmup seed: memset BEFORE the TileContext so the first
    # warmup matmul issues right after the preamble barrier; the clock ramp
    # then completes before the first data-gated matmul.
    wz = nc.alloc_sbuf_tensor("wz0", [128, 128], _BF16)
    nc.vector.memset(wz.ap(), 0.0)

    # weighted evacuation rotation: error-diffusion on per-copy engine time
    # (DVE + Activation only: GPSIMD cannot access PSUM on TRN2)
    _rates = {"A": 1.0 / 996.0, "D": 1.0 / 1192.0}
    _tot = sum(_rates.values())
    _shares = {k: v / _tot for k, v in _rates.items()}

    with tile.TileContext(nc) as tc:
        with (
            tc.tile_pool(name="consts", bufs=1) as cpool,
            tc.tile_pool(name="outp", bufs=6) as outpool,
            tc.tile_pool(name="ps", bufs=4, space="PSUM") as ps,
        ):
            w1_t = cpool.tile([128, 128], _BF16, tag="w1")
            # SWDGE path: keeps w1's desc-gen off the shared HWDGE so the
            # first input chunk's gen starts immediately
            nc.gpsimd.dma_start(w1_t[:], w1_d.ap())

            in_t = cpool.tile([128, 256, 128], in_dt, tag="in")
            w2_t = cpool.tile([128, 32, 2, 128], _BF16, tag="w2")

            def in_chunks(splits, lo):
                for ch in splits:
                    nc.sync.dma_start(in_t[:, lo:lo + ch],
                                      in_d.ap()[:, lo:lo + ch])
                    lo += ch
                return lo

            def w2_chunks():
                # each 4-block chunk unblocks 4 kap2 iterations (PI order)
                for e in range(8):
                    nc.sync.dma_start(w2_t[:, 4 * e:4 * (e + 1)],
                                      w2_d.ap()[:, 4 * e:4 * (e + 1)])

            if split_b:
                # w2 mid-ladder: early enough for the interleaved stage-B
                # first half, late enough not to starve the PE's first groups
                lo = in_chunks(IN_SPLITS[:5], 0)
                w2_chunks()
                in_chunks(IN_SPLITS[5:], lo)
            else:
                # stage B begins only after all of stage A: w2 can trail
                in_chunks(IN_SPLITS, 0)
                w2_chunks()

            # PE p-state warmup: bridge the input-DMA latency with throwaway
            # matmuls so every real matmul runs at the full 2.4GHz clock.
            wb = ps.tile([128, 1024], _F32, tag="ps")
            for w in range(NWARM):
                nc.tensor.matmul(wb[:, :64], wz.ap(), wz.ap()[:, :64],
                                 start=True, stop=True)

            # preload the Activation copy table during the DMA wait
            warm_sb = cpool.tile([128, 1], _BF16, tag="actwarm")
            nc.scalar.copy(warm_sb[:], wz.ap()[:, :1])

            _err = dict.fromkeys(_rates, 0.0)

            def evac(out_ap, in_ap):
                for k in _err:
                    _err[k] += _shares[k]
                eng = max(_err, key=lambda k: _err[k])
                _err[eng] -= 1.0
                if eng == "D":
                    nc.vector.tensor_copy(out_ap, in_ap)
                else:
                    nc.scalar.copy(out_ap, in_ap)

            # ---- stage A: 256 data-stationary one-pass matmuls ----
            # T cols, flattened: (g, dm, q, kap2, u) — each bank lands
            # contiguously; stage-B slices are then b-ordered.
            T = cpool.tile([128, 64, 4, 2, 32, 2], _BF16, tag="T")
            Tf = T.rearrange("p a b c d e -> p (a b c d e)")

            def a_pair(gp):
                # two 4-m groups share one 2-bank PSUM tile: a single
                # spanning evacuation halves the per-instruction PSUM-access
                # overhead on the (binding) evac engines. The final pair
                # instead uses two independent tiles whose evacs rotate onto
                # both engines, halving the last-evac latency gating stage B.
                if gp == 31:
                    for half in range(2):
                        g = 2 * gp + half
                        bank = ps.tile([128, 512], _F32, tag="ps")
                        for dm in range(4):
                            m = 4 * g + dm
                            nc.tensor.matmul(
                                bank[:, 128 * dm:128 * (dm + 1)],
                                in_t[:, m], w1_t[:], start=True, stop=True)
                        evac(Tf[:, 512 * g:512 * (g + 1)], bank[:])
                    return
                bank = ps.tile([128, 1024], _F32, tag="ps")
                for dm in range(8):
                    m = 8 * gp + dm
                    nc.tensor.matmul(bank[:, 128 * dm:128 * (dm + 1)],
                                     in_t[:, m], w1_t[:],
                                     start=True, stop=True)
                evac(Tf[:, 1024 * gp:1024 * (gp + 1)], bank[:])

            wn_t = cpool.tile([128, 32, 128], _BF16, tag="wn")
            _negated = set()

            def negate_chunk(e):
                # -Bi for blocks [4e, 4e+4): DVE/ACT alternate; emitted just
                # before first use so they slot behind the stage-A evacs
                if e in _negated:
                    return
                _negated.add(e)
                dst = wn_t[:, 4 * e:4 * (e + 1)]
                srcn = w2_t[:, 4 * e:4 * (e + 1), 1]
                if e % 2 == 0:
                    nc.vector.tensor_scalar_mul(dst, srcn, -1.0)
                else:
                    nc.scalar.mul(dst, srcn, -1.0)

            def b_weights(s, kap2):
                br = w2_t[:, s, 0]
                bi = w2_t[:, s, 1]
                bni = wn_t[:, s]
                return br, bi, bni

            def b_half(s, kap2, h, wide):
                gs = slice(0, 64) if wide else slice(32 * h, 32 * h + 32)
                nb = 512 if wide else 256
                tr = T[:, gs, :, 0, kap2, :]
                ti = T[:, gs, :, 1, kap2, :]
                br, bi, bni = b_weights(s, kap2)
                yrz = ps.tile([128, 2 * nb], _F32, tag="ps")
                yr = yrz[:, 0:nb]
                yi = yrz[:, nb:2 * nb]
                nc.tensor.matmul(yr, br, tr, start=True, stop=False)
                nc.tensor.matmul(yr, bni, ti, start=False, stop=True)
                nc.tensor.matmul(yi, bi, tr, start=True, stop=False)
                nc.tensor.matmul(yi, br, ti, start=False, stop=True)
                ot = outpool.tile([128, 2, nb], _BF16, tag="out")
                otf = ot.rearrange("p a b -> p (a b)")
                if wide and s == len(PI) - 1:
                    # final-kap2 latency chain: evacuate the planes on both
                    # engines in parallel and ship each as its own DMA on a
                    # different queue so gens and transfers overlap
                    nc.vector.tensor_copy(ot[:, 0], yr)
                    nc.gpsimd.dma_start(out_d[kap2][:, 0], ot[:, 0])
                    nc.scalar.copy(ot[:, 1], yi)
                    nc.sync.dma_start(out_d[kap2][:, 1], ot[:, 1])
                    return
                # (re | im) blocks of yrz match ot's (pl, b) layout: one
                # spanning evacuation covers the whole kap2 output
                evac(otf[:], yrz[:])
                if wide:
                    nc.sync.dma_start(out_d[kap2], ot[:])
                elif h == 0:
                    # first-half outs ride SWDGE (Pool) — keeps the 32 extra
                    # desc-gens off the serialized HWDGE device
                    nc.gpsimd.dma_start(out_d[kap2][:, :, 0:256], ot[:])
                else:
                    nc.sync.dma_start(out_d[kap2][:, :, 256:512], ot[:])

            if not split_b:
                for gp in range(32):
                    a_pair(gp)
                for s, kap2 in enumerate(PI):
                    negate_chunk(s // 4)
                    b_half(s, kap2, 0, True)
            else:
                # interleave: stage-B's first batch-half rides along as soon
                # as the first half of T exists, filling the PE while stage A
                # is evacuation-paced
                for gp in range(16):
                    a_pair(gp)
                for s, kap2 in enumerate(PI):
                    negate_chunk(s // 4)
                    b_half(s, kap2, 0, False)
                    if s < 16:
                        a_pair(16 + s)
                for s, kap2 in enumerate(PI):
                    b_half(s, kap2, 1, False)

    nc.finalize()
    _nc_cache[in_dt] = nc
    return nc


# ---------------------------------------------------------------------------
# launch helper
# ---------------------------------------------------------------------------

_last_exec_ns = None


def last_exec_time_ns():
    """Sum of HW exec times (ns) of the launches in the last kernel() call,
    when KERNEL_TRACE=1 was set and NTFF profiling is available. None otherwise."""
    return _last_exec_ns


def predicted_exec_time_ns():
    """Cost-model (TimelineSim) predicted HW exec time for both launches, ns."""
    from concourse.timeline_sim import TimelineSim
    total = 0
    for dt_ in (_BF16, _FP8):
        total += int(TimelineSim(_build_nc(dt_)).simulate())
    return total


def _run_launch(cols_re, cols_im, in_dt, npdt, w1_scale):
    """cols_re/cols_im: list of 8 planes [4096 f][512 b] f32.
    Returns list of 8 (Fre, Fim) planes [4096 k][512 b]."""
    global _last_exec_ns
    import os
    nc = _build_nc(in_dt)
    w1, w2 = _make_consts()
    w1s = (w1 * w1_scale).astype(_NPBF16)
    in_maps = []
    for c in range(NCORES):
        in_maps.append({
            "in2": _marshal_in(cols_re[c], cols_im[c], npdt),
            "w1": w1s, "w2": w2,
        })
    trace = bool(os.environ.get("KERNEL_TRACE"))
    try:
        res = run_bass_kernel_spmd(nc, in_maps, core_ids=list(range(NCORES)),
                                   trace=trace)
    except ModuleNotFoundError:
        res = run_bass_kernel_spmd(nc, in_maps, core_ids=list(range(NCORES)))
    if trace and getattr(res, "exec_time_ns", None) is not None:
        _last_exec_ns = (_last_exec_ns or 0) + res.exec_time_ns
    return [_unmarshal_out(res.results[c]["out2"]) for c in range(NCORES)]


# ---------------------------------------------------------------------------
# public entry point
# ---------------------------------------------------------------------------

def kernel(x: np.ndarray) -> np.ndarray:
    """x: [N, 2] float32 (re, im). Returns FFT(x) as [N, 2] float32."""
    global _last_exec_ns
    _last_exec_ns = None
    x = np.asarray(x)
    Are = np.ascontiguousarray(x[:, 0].reshape(NG, NG))  # [j2g][j1g]
    Aim = np.ascontiguousarray(x[:, 1].reshape(NG, NG))

    # launch 1 (bf16 input): FFT over rows (j2g) for each column j1g
    cols_re = [np.ascontiguousarray(Are[:, BPC * c:BPC * (c + 1)]) for c in range(NCORES)]
    cols_im = [np.ascontiguousarray(Aim[:, BPC * c:BPC * (c + 1)]) for c in range(NCORES)]
    l1 = _run_launch(cols_re, cols_im, _BF16, _NPBF16, 1.0)

    # host: assemble F [k2g][j1g], twiddle, transpose-exchange
    F = np.empty((NG, NG), np.complex64)
    for c in range(NCORES):
        fre, fim = l1[c]
        F[:, BPC * c:BPC * (c + 1)] = fre + 1j * fim
    F *= _global_twiddle()

    # launch 2 (fp8 input): FFT over j1g for each row k2g; rescale to
    # FP8_SCALE/sigma for e3m4, undo via the stage-A weights.
    sigma = float(np.sqrt(np.mean(F.real.astype(np.float64) ** 2
                                  + F.imag.astype(np.float64) ** 2) / 2.0))
    s2 = FP8_SCALE / sigma
    cols_re2 = []
    cols_im2 = []
    for d in range(NCORES):
        block = F[BPC * d:BPC * (d + 1), :].T      # [j1g][k2g-local]
        cols_re2.append(np.ascontiguousarray(block.real) * np.float32(s2))
        cols_im2.append(np.ascontiguousarray(block.imag) * np.float32(s2))
    l2 = _run_launch(cols_re2, cols_im2, _FP8, _NPFP8, 1.0 / s2)

    # assemble Xmat [k1g][k2g]; out flat index k = 4096*k1g + k2g
    out = np.empty((NG, NG, 2), np.float32)
    for d in range(NCORES):
        rre, rim = l2[d]
        out[:, BPC * d:BPC * (d + 1), 0] = rre
        out[:, BPC * d:BPC * (d + 1), 1] = rim
    return out.reshape(N, 2)
